# revision 8
# baseline (speedup 1.0000x reference)
"""GraphConv x3 + segment_max pooling for Trainium2 (8 NeuronCores).

Single-launch design, dst-sharded:
  - Edges sorted by dst; core k owns dst nodes [25000k, 25000(k+1)).
  - Gather streams by src core: GpSimd ap_gather stream j reads the z-table
    slab of core j's nodes ([16ch x 25000] fp32 per 16-partition group,
    [128, 25024] total, SBUF resident).
  - Per (stream, 512-dst-node chunk): gather src values in dst order (fp32),
    DVE prefix scan, boundary ap_gather at per-node run ends, adjacent diff
    = per-(node, stream) sums.  PE matmul vs a tiled selection/weight matrix
    sums the 8 streams and applies the layer weight in one shot.
  - Root term via small PE matmuls; bias+relu fused on the Activation engine.
  - Between layers: z_{l+1} = W_rel @ h_l per own-node chunk (PE), DMA to an
    internal DRAM slab [16, 25000]; HW AllGather -> [128, 25000] = next table.
  - Host does edge bucketing (vectorized numpy) and the final segment_max +
    linear head (tiny).
Falls back to a pure-numpy reference implementation on any device failure.
"""
import sys
import time

import numpy as np

N_NODES = 200_000
N_EDGES = 6_400_000
N_GRAPHS = 512
NC = 8
NPC = N_NODES // NC          # 25000 dst nodes per core
TBL = 25024                  # gather table cols (25000 + zero pad)
PADCOL = 25000               # guaranteed-zero table column
CH = 512                     # dst nodes per chunk
NCH = (NPC + CH - 1) // CH   # 49 chunks
NCHX = NCH * CH              # 25088 padded node slots
NBW = ((CH + 1 + 15) // 16) * 16   # boundary idxs per chunk = 528


def _wrap16(a):
    """[..., 8, n] -> [..., 128, n//16] int16 ap_gather wrapped layout."""
    *lead, eight, n = a.shape
    out = np.zeros((*lead, 128, n // 16), np.int16)
    a4 = a.reshape(*lead, 8, n // 16, 16)
    for j in range(8):
        out[..., 16 * j:16 * j + 16, :] = np.swapaxes(a4[..., j, :, :], -1, -2)
    return out


def _build_indices(src, dst):
    """Per-core wrapped gather/boundary index tensors. Returns (list, NI)."""
    order = np.argsort(dst, kind="stable")
    s_s = src[order].astype(np.int32)
    d_s = dst[order].astype(np.int32)
    core_bounds = np.searchsorted(d_s, np.arange(NC + 1) * NPC)
    cores = []
    for k in range(NC):
        lo, hi = core_bounds[k], core_bounds[k + 1]
        sk = s_s[lo:hi]
        dk = d_s[lo:hi] - k * NPC
        jj = sk // NPC                     # stream = src core
        col = sk % NPC                     # table column
        bucket = (dk // CH) * 8 + jj
        o2 = np.argsort(bucket, kind="stable")
        bucket_s = bucket[o2]
        col_s = col[o2]
        counts = np.bincount(bucket_s, minlength=NCH * 8)
        starts = np.concatenate([[0], np.cumsum(counts)[:-1]])
        pos = np.arange(len(bucket_s)) - starts[bucket_s]
        cores.append((counts, bucket_s, pos, col_s, dk, jj))
    ni = max(int(c[0].max()) for c in cores)
    NI = ((ni + 15) // 16) * 16
    if NI + 1 > 32000:
        raise RuntimeError(f"chunk overflow: NI={NI}")
    out = []
    for k, (counts, bucket_s, pos, col_s, dk, jj) in enumerate(cores):
        eidx = np.full((NCH * 8, NI), PADCOL, np.int16)
        eidx[bucket_s, pos] = col_s.astype(np.int16)
        hist = np.zeros((8, NCHX), np.int64)
        np.add.at(hist, (jj, dk), 1)
        ends = np.cumsum(hist.reshape(8, NCH, CH), axis=2)
        bidx = np.zeros((NCH, 8, NBW), np.int16)
        bidx[:, :, 1:CH + 1] = np.swapaxes(ends, 0, 1).astype(np.int16)
        out.append((_wrap16(eidx.reshape(NCH, 8, NI)), _wrap16(bidx)))
    return out, NI


def _np_reference(x, src, dst, batch, Ws):
    (W1r, b1, W1o, W2r, b2, W2o, W3r, b3, W3o, Wl, bl) = Ws

    def conv(h, Wr, b, Wo):
        agg = np.zeros((N_NODES, h.shape[1]), np.float32)
        np.add.at(agg, dst, h[src])
        return np.maximum(agg @ Wr.T + b + h @ Wo.T, 0.0)

    h = conv(x, W1r, b1, W1o)
    h = conv(h, W2r, b2, W2o)
    h = conv(h, W3r, b3, W3o)
    return _head(h, batch, Wl, bl)


def _head(h3, batch, Wl, bl):
    bounds = np.searchsorted(batch, np.arange(N_GRAPHS + 1))
    pooled = np.full((N_GRAPHS, 8), -np.inf, np.float32)
    ne = np.nonzero(bounds[1:] > bounds[:-1])[0]
    if len(ne):
        red = np.maximum.reduceat(h3, bounds[ne], axis=0)
        pooled[ne] = red[:len(ne)]
    return (pooled @ Wl.T + bl)[:, 0].astype(np.float32)


def _patch_tile():
    """Split multi-wait sync_info into single-wait NoOps/Drains (walrus
    codegen supports one sync wait per instruction)."""
    import bass_rust
    import concourse.mybir as mybir
    import concourse.tile as tilemod
    from concourse.vector_clock import ScopedClock

    if getattr(tilemod, "_kernel_wait_patch", False):
        return
    tilemod._kernel_wait_patch = True

    def _split_waits(nc, ordered):
        for _bb, insts in ordered.items():
            new_list = []
            for inst in insts:
                si = inst.sync_info
                waits = list(si.on_wait) if si is not None and si.on_wait else []
                if len(waits) > 1:
                    for w in waits[:-1]:
                        nop = bass_rust.InstNoOp(
                            name=f"I-{nc.next_id()}-waitnop", ins=[], outs=[])
                        nop.engine = inst.engine
                        nop.sync_info = mybir.SyncInfo(on_wait=[w], on_update=[])
                        nc.register_instruction(nop, overwrite=True)
                        new_list.append(nop)
                    inst.sync_info = mybir.SyncInfo(
                        on_wait=waits[-1:],
                        on_update=list(si.on_update) if si.on_update else [])
                new_list.append(inst)
            insts[:] = new_list
        return ordered

    orig_lower = tilemod.TileContext._lower_ordered_insts

    def _lower_ordered_insts(self, ordered):
        _split_waits(self.nc, ordered)
        return orig_lower(self, ordered)

    tilemod.TileContext._lower_ordered_insts = _lower_ordered_insts

    def _drain_and_barrier(self, tick_clock, wait_clock):
        drain_inst = self.nc.sync.drain()
        wait_clock.add_sem_waits(
            drain_inst.ins, ScopedClock({None: tick_clock.global_clock}))
        si = drain_inst.ins.sync_info
        waits = list(si.on_wait) if si is not None and si.on_wait else []
        if len(waits) > 1:
            drain_inst.ins.sync_info = mybir.SyncInfo(
                on_wait=waits[:1],
                on_update=list(si.on_update) if si.on_update else [])
            for i in range(1, len(waits)):
                d2 = self.nc.sync.drain()
                d2.ins.sync_info = mybir.SyncInfo(
                    on_wait=waits[i:i + 1], on_update=[])
        self.nc.all_engine_barrier()
        assert self.sems is not None
        popped = self.nc._tile_sem_poison_stack.pop()
        assert popped is self._sem_poison
        self.nc.clear_and_free_semaphores(list(self.sems.allocated().values()))
        self.nc.all_engine_barrier()

    tilemod.TileContext._drain_and_barrier = _drain_and_barrier


def _build_bass(NI, iters=1):
    sys.path.insert(0, "/opt/trn_rl_repo")
    import concourse.bass as bass
    import concourse.mybir as mybir
    import concourse.tile as tile
    from concourse import library_config
    from concourse.library_overlay import lower_extended_insts

    _patch_tile()
    f32, bf16, i16 = mybir.dt.float32, mybir.dt.bfloat16, mybir.dt.int16
    RELU = mybir.ActivationFunctionType.Relu
    ADD = mybir.AluOpType.add
    SUB = mybir.AluOpType.subtract

    nc = bass.Bass("TRN2", num_devices=NC)
    xtab_h = nc.dram_tensor("xtab", [4, N_NODES], f32, kind="ExternalInput")
    xr_h = nc.dram_tensor("xr", [16, NPC], bf16, kind="ExternalInput")
    eidx_h = nc.dram_tensor("eidx", [NCH, 128, NI // 16], i16,
                            kind="ExternalInput")
    bidx_h = nc.dram_tensor("bidx", [NCH, 128, NBW // 16], i16,
                            kind="ExternalInput")
    wra1_h = nc.dram_tensor("wra1", [128, 32], bf16, kind="ExternalInput")
    sel16_h = nc.dram_tensor("sel16", [128, 16], bf16, kind="ExternalInput")
    sel8_h = nc.dram_tensor("sel8", [128, 8], bf16, kind="ExternalInput")
    w1o_h = nc.dram_tensor("w1o", [16, 32], bf16, kind="ExternalInput")
    w2o_h = nc.dram_tensor("w2o", [32, 16], bf16, kind="ExternalInput")
    w3o_h = nc.dram_tensor("w3o", [16, 8], bf16, kind="ExternalInput")
    w2r_h = nc.dram_tensor("w2r", [32, 16], bf16, kind="ExternalInput")
    w3r_h = nc.dram_tensor("w3r", [16, 16], bf16, kind="ExternalInput")
    b1_h = nc.dram_tensor("b1", [32, 1], f32, kind="ExternalInput")
    b2_h = nc.dram_tensor("b2", [16, 1], f32, kind="ExternalInput")
    b3_h = nc.dram_tensor("b3", [8, 1], f32, kind="ExternalInput")
    hout_h = nc.dram_tensor("hout", [8, NPC], f32, kind="ExternalOutput")

    with tile.TileContext(nc) as tc:
        nc.gpsimd.load_library(library_config.ap_gather)
        with tc.tile_pool(name="c", bufs=1) as cp, \
             tc.tile_pool(name="p", bufs=2) as pool, \
             tc.tile_pool(name="pb", bufs=2, space="PSUM") as psb, \
             tc.tile_pool(name="pc", bufs=2, space="PSUM") as psc, \
             tc.tile_pool(name="dr", bufs=1, space="DRAM") as dp:
            tab = cp.tile([128, TBL], f32)
            zbig = cp.tile([128, NI], bf16)
            wra1 = cp.tile([128, 32], bf16)
            sel16 = cp.tile([128, 16], bf16)
            sel8 = cp.tile([128, 8], bf16)
            w1o = cp.tile([16, 32], bf16)
            w2o = cp.tile([32, 16], bf16)
            w3o = cp.tile([16, 8], bf16)
            w2r = cp.tile([32, 16], bf16)
            w3r = cp.tile([16, 16], bf16)
            b1 = cp.tile([32, 1], f32)
            b2 = cp.tile([16, 1], f32)
            b3 = cp.tile([8, 1], f32)
            for t, h in ((wra1, wra1_h), (sel16, sel16_h), (sel8, sel8_h),
                         (w1o, w1o_h), (w2o, w2o_h), (w3o, w3o_h),
                         (w2r, w2r_h), (w3r, w3r_h),
                         (b1, b1_h), (b2, b2_h), (b3, b3_h)):
                nc.sync.dma_start(t[:], h[:, :])
            nc.vector.memset(zbig[:], 0.0)
            nc.vector.memset(tab[:], 0.0)

            def r3(ap, n):
                return ap.rearrange("p (n d) -> p n d", d=1)

            for _it in range(iters):
                h1s = dp.tile([32, NCHX], bf16, tag="h1s")
                h2s = dp.tile([16, NCHX], bf16, tag="h2s")
                for j in range(NC):
                    nc.sync.dma_start(tab[16 * j:16 * j + 4, :NPC],
                                      xtab_h[:, j * NPC:(j + 1) * NPC])
                for L, (wra, wo, bias, Cout, Crt, hsrc, wnext, hdst) in \
                        enumerate((
                        (wra1, w1o, b1, 32, 16, None, w2r, h1s),
                        (sel16, w2o, b2, 16, 32, h1s, w3r, h2s),
                        (sel8, w3o, b3, 8, 16, h2s, None, None))):
                    zown = (dp.tile([16, NPC], f32, tag="zown", name="zown")
                            if L < 2 else None)
                    for m in range(NCH):
                        n = min(CH, NPC - m * CH)
                        et = pool.tile([128, NI // 16], i16, tag="et")
                        nc.sync.dma_start(et[:], eidx_h[m, :, :])
                        bt = pool.tile([128, NBW // 16], i16, tag="bt")
                        nc.sync.dma_start(bt[:], bidx_h[m, :, :])
                        g = pool.tile([128, NI], f32, tag="g")
                        nc.gpsimd.ap_gather(
                            out_ap=r3(g[:], NI), in_ap=r3(tab[:], TBL),
                            idxs_ap=et[:], channels=128, num_elems=TBL,
                            d=1, num_idxs=NI)
                        s = pool.tile([128, NI + 16], f32, tag="s")
                        nc.scalar.memzero(s[:, 0:1])
                        nc.vector.tensor_tensor_scan(
                            s[:, 1:NI + 1], g[:], zbig[:], 0.0,
                            op0=ADD, op1=ADD)
                        bnd = pool.tile([128, NBW], f32, tag="bnd")
                        nc.gpsimd.ap_gather(
                            out_ap=r3(bnd[:], NBW),
                            in_ap=r3(s[:, :NI + 1], NI + 1),
                            idxs_ap=bt[:], channels=128, num_elems=NI + 1,
                            d=1, num_idxs=NBW)
                        dift = pool.tile([128, CH], bf16, tag="dift")
                        nc.vector.tensor_tensor(
                            dift[:], bnd[:, 1:CH + 1], bnd[:, :CH], op=SUB)
                        pB = psb.tile([Cout, CH], f32, tag="pB")
                        nc.tensor.matmul(pB[:], wra[:], dift[:],
                                         start=True, stop=False)
                        hr = pool.tile([32, CH], bf16, tag="hr")
                        if L == 0:
                            nc.sync.dma_start(hr[:16, :n],
                                              xr_h[:, m * CH:m * CH + n])
                        else:
                            nc.sync.dma_start(
                                hr[:Crt, :], hsrc[:, m * CH:(m + 1) * CH])
                        rt = hr[:Crt, :]
                        nc.tensor.matmul(pB[:], wo[:], rt,
                                         start=False, stop=True)
                        if L < 2:
                            hch = pool.tile([Cout, CH], bf16, tag="hch")
                            nc.scalar.activation(hch[:], pB[:], RELU,
                                                 bias=bias[:, 0:1])
                            nc.sync.dma_start(
                                hdst[:, m * CH:(m + 1) * CH], hch[:])
                            pC = psc.tile([16, CH], f32, tag="pC")
                            nc.tensor.matmul(pC[:], wnext[:], hch[:],
                                             start=True, stop=True)
                            zch = pool.tile([16, CH], f32, tag="zch")
                            nc.scalar.copy(zch[:], pC[:])
                            nc.sync.dma_start(zown[:, m * CH:m * CH + n],
                                              zch[:, :n])
                        else:
                            ho = pool.tile([8, CH], f32, tag="ho")
                            nc.scalar.activation(ho[:], pB[:], RELU,
                                                 bias=bias[:, 0:1])
                            nc.sync.dma_start(hout_h[:, m * CH:m * CH + n],
                                              ho[:, :n])
                    if L < 2:
                        zfull = dp.tile([128, NPC], f32, tag="zfull")
                        nc.gpsimd.collective_compute(
                            "AllGather", mybir.AluOpType.bypass,
                            replica_groups=[list(range(NC))],
                            ins=[zown[:].opt()], outs=[zfull[:].opt()])
                        nc.sync.dma_start(tab[:, :NPC], zfull[:])
    lower_extended_insts(nc)
    return nc


def _prepare(inputs):
    import ml_dtypes
    bf = ml_dtypes.bfloat16
    x = np.asarray(inputs["x"], np.float32)
    ei = np.asarray(inputs["edge_index"])
    src = ei[0].astype(np.int64)
    dst = ei[1].astype(np.int64)
    batch = np.asarray(inputs["batch"]).astype(np.int64)
    W = {k: np.asarray(inputs[k], np.float32) for k in
         ("W1_rel", "b1", "W1_root", "W2_rel", "b2", "W2_root",
          "W3_rel", "b3", "W3_root", "W_lin", "b_lin")}
    idx, NI = _build_indices(src, dst)

    xtab = np.zeros((4, N_NODES), np.float32)
    xtab[:3] = x.T
    wra1 = np.zeros((128, 32), np.float32)
    wra1[:, :] = np.tile(
        np.concatenate([W["W1_rel"].T, np.zeros((13, 32), np.float32)], 0),
        (8, 1))
    sel16 = np.tile(np.eye(16, dtype=np.float32), (8, 1))
    sel8 = np.tile(np.eye(16, 8, dtype=np.float32), (8, 1))
    w1o = np.concatenate([W["W1_root"].T, np.zeros((13, 32), np.float32)], 0)
    w2o = W["W2_root"].T
    w3o = W["W3_root"].T
    w2r = W["W2_rel"].T
    w3r = np.concatenate([W["W3_rel"].T, np.zeros((16, 8), np.float32)], 1)

    shared = {
        "xtab": xtab,
        "wra1": wra1.astype(bf), "sel16": sel16.astype(bf),
        "sel8": sel8.astype(bf),
        "w1o": w1o.astype(bf), "w2o": w2o.astype(bf), "w3o": w3o.astype(bf),
        "w2r": w2r.astype(bf), "w3r": w3r.astype(bf),
        "b1": W["b1"].reshape(32, 1), "b2": W["b2"].reshape(16, 1),
        "b3": W["b3"].reshape(8, 1),
    }
    in_maps = []
    for k in range(NC):
        xr = np.zeros((16, NPC), bf)
        xr[:3] = x[k * NPC:(k + 1) * NPC].T.astype(bf)
        m = dict(shared)
        m["xr"] = xr
        m["eidx"], m["bidx"] = idx[k]
        in_maps.append(m)
    return dict(in_maps=in_maps, NI=NI, batch=batch,
                Wl=W["W_lin"], bl=W["b_lin"],
                x=x, src=src, dst=dst,
                Ws=(W["W1_rel"], W["b1"], W["W1_root"], W["W2_rel"], W["b2"],
                    W["W2_root"], W["W3_rel"], W["b3"], W["W3_root"],
                    W["W_lin"], W["b_lin"]))


def _execute(nc, prep):
    from concourse.bass_utils import run_bass_kernel_spmd
    t0 = time.time()
    res = run_bass_kernel_spmd(nc, prep["in_maps"], core_ids=list(range(NC)))
    wall = time.time() - t0
    h3 = np.concatenate([r["hout"].T for r in res.results], 0)
    return h3, wall


def kernel(**inputs):
    prep = _prepare(inputs)
    try:
        nc = _build_bass(prep["NI"], iters=1)
        h3, wall = _execute(nc, prep)
        if not np.isfinite(h3).all():
            raise RuntimeError("non-finite device output")
        kernel.last_hw_s = wall
        return _head(h3, prep["batch"], prep["Wl"], prep["bl"])
    except Exception as e:  # pragma: no cover - device fallback
        print(f"[kernel] device path failed ({type(e).__name__}: {e}); "
              f"falling back to numpy", file=sys.stderr)
        kernel.last_hw_s = -1.0
        return _np_reference(prep["x"], prep["src"], prep["dst"],
                             prep["batch"], prep["Ws"]).astype(np.float32)
